# revision 1
# baseline (speedup 1.0000x reference)
"""Trainium2 Bass kernel for nn_Model_39676907886571 (per-head attention, S=2048, d=3).

Math (per head h, fully head/data parallel, one head per NeuronCore):
  q_mat = query[h] @ x[h].T          (3, S)   -> q = q_mat viewed row-major as (S, 3)
  k_mat, v_mat likewise (the reshape is a memory-reinterpreting view, not a transpose)
  attn  = softmax(q @ k.T / sqrt(3)) (S, S)
  out   = (attn @ v).T               (3, S)

Device strategy (all on-chip, the S x S attention matrix never touches HBM):
  * qkv = W9 @ xT on the PE; a DRAM bounce reshapes the row-major flats into the
    "natural" (S, 3) triple layout that the weird view demands.
  * q^T / k^T are rebuilt by 16 PE transposes each, which produce a "u-order"
    permutation of the sequence axis (u = 128*c + p  <->  t_true = 16*p + c).
    Softmax sums over the key axis are permutation-invariant; the query axis is
    un-permuted at the end by a strided DVE write fused into the normalization.
  * E^T = exp(k-chunks^T @ q^T / sqrt(3)) keeps the key axis on partitions, so
    attn @ [1|v] needs no transposes and the softmax denominator falls out of
    the ones column of the [1|v] stationary operand.
  * Matmul operands are float32r (single-pass fp32 PE mode, 4x faster than the
    fp32hi/lo pair); PSUM ping-pong is managed manually so consecutive rounds
    only serialize through the exp (ACT is the bottleneck engine).
  * PSUM budget (8 banks): 2 x 3-bank ping-pong E^T tiles + 2 x 1-bank
    attn@[1|v] accumulators (per-s-chunk, accumulated across rounds in PSUM so
    no DVE op sits on the mm1 -> exp critical cycle).  The main loop is
    software-pipelined: mm1 of round g+1 issues before mm2 of round g so the
    in-order PE stream overlaps the exp; leftover q/k transpose groups ride
    idle PE slots mid-loop, writing into idle PSUM accumulators (dead until
    their first start=True matmul) so they never collide with live data.
    Cost model (TimelineSim): ~53 us/core, ACT-saturated (exp of the 2048^2
    attention matrix = 34 us floor at 128 lanes / 1.2 GHz).
"""

import numpy as np
from contextlib import ExitStack

import concourse.bass as bass
import concourse.tile as tile
from concourse import bacc, mybir
from concourse import bass_utils

F32 = mybir.dt.float32
F32R = mybir.dt.float32r

H, S, D = 8, 2048, 3
NCH = 16                # t-chunks of 128 (u-order blocks)
SQ = 512                # s-chunk width (one PSUM bank)
INV_SCALE = float(1.0 / np.sqrt(3.0))


def _r(ap):
    """Bitcast an fp32 AP to float32r (same bits)."""
    return ap.bitcast(F32R)


def build_program(reps=1):
    nc = bacc.Bacc("TRN2", num_devices=H, debug=False)
    xt_dram = nc.dram_tensor("xt", (3, S), F32, kind="ExternalInput")
    wt_dram = nc.dram_tensor("wt", (3, 9), F32, kind="ExternalInput")
    out_dram = nc.dram_tensor("out", (3, S), F32, kind="ExternalOutput")
    scratch = nc.dram_tensor("scratch", (3, 3 * S), F32, kind="Internal")

    with tile.TileContext(nc) as tc, ExitStack() as ctx:
        consts = ctx.enter_context(tc.tile_pool(name="consts", bufs=1))
        sb = ctx.enter_context(tc.tile_pool(name="sb", bufs=2 if reps > 1 else 1))
        es = ctx.enter_context(tc.tile_pool(name="es", bufs=4))
        ping = ctx.enter_context(tc.tile_pool(name="ping", bufs=1, space="PSUM"))
        accp = ctx.enter_context(tc.tile_pool(name="accp", bufs=1, space="PSUM"))

        # constants (shared across reps)
        ident_f = consts.tile([128, 128], F32)
        from concourse.masks import make_identity

        make_identity(nc, ident_f)
        ident = consts.tile([128, 128], F32R)
        nc.vector.tensor_copy(ident[:], ident_f[:])
        onesq_f = consts.tile([128, 16], F32)
        nc.vector.memset(onesq_f, 1.0)
        onesq = consts.tile([128, 16], F32R)
        nc.vector.tensor_copy(onesq[:], onesq_f[:])
        ones4 = consts.tile([1, 4], F32R)
        nc.vector.tensor_copy(ones4[:], onesq_f[0:1, 0:4])
        # prewarm the ACT exp table so the ~2.7us table load overlaps the prologue
        warm = consts.tile([1, 1], F32)
        nc.scalar.activation(warm[:], onesq_f[0:1, 0:1], mybir.ActivationFunctionType.Exp)

        for _rep in range(reps):
            _build_body(nc, tc, sb, es, ping, accp, ident, onesq, ones4,
                        xt_dram, wt_dram, out_dram, scratch)

    nc.compile()
    return nc


def _build_body(nc, tc, sb, es, ping, accp, ident, onesq, ones4,
                xt_dram, wt_dram, out_dram, scratch):
    psA = ping.tile([128, 3 * SQ], F32, tag="A")
    psB = ping.tile([128, 3 * SQ], F32, tag="B")
    pst_of = lambda g: psA if g % 2 == 0 else psB

    wT_sb = sb.tile([3, 9], F32R)
    nc.scalar.dma_start(wT_sb[:], _r(wt_dram.ap()))
    xT = sb.tile([3, S], F32R)
    nc.sync.dma_start(xT[:, 0 : 2 * SQ], _r(xt_dram.ap()[:, 0 : 2 * SQ]))
    nc.scalar.dma_start(xT[:, 2 * SQ : S], _r(xt_dram.ap()[:, 2 * SQ : S]))

    # qkv = W9 @ xT  (9, S), true t-order; PSUM -> SBUF -> DRAM bounce -> nats
    for m in range(4):
        tgt = psA[0:9, SQ * m : SQ * (m + 1)] if m < 3 else psB[0:9, 0:SQ]
        nc.tensor.matmul(
            tgt,
            lhsT=wT_sb[:],
            rhs=xT[:, SQ * m : SQ * (m + 1)],
            start=True,
            stop=True,
        )
    qkv_sb = sb.tile([9, S], F32)
    nc.scalar.copy(qkv_sb[:, 0 : 3 * SQ], psA[0:9, :])
    nc.vector.tensor_copy(qkv_sb[:, 3 * SQ : S], psB[0:9, 0:SQ])

    # warm the PE pstate during the otherwise idle DMA-bounce window so the
    # first transposes/matmuls run at full clock (writes are dead; mm1(0)
    # overwrites the same PSUM region later)
    for _w in range(10):
        nc.tensor.transpose(_r(psB[0:128, SQ : SQ + 128]), ident[:], ident[:])

    # natural (S, 3)-triple layout via a DRAM bounce (partition-crossing
    # reshape); per-tensor stores/loads pipelined across the two HWDGE queues
    nats = sb.tile([128, 144], F32R)
    scr = scratch.ap()
    nc.sync.dma_start(scr[0, :], qkv_sb[0:3, :])
    nc.scalar.dma_start(scr[1, :], qkv_sb[3:6, :])
    nc.scalar.dma_start(nats[:, 0:48], _r(scr[0, :]))
    nc.sync.dma_start(nats[:, 48:96], _r(scr[1, :]))
    # v's bounce is issued after the q/k loads: HWDGE descriptor generation is
    # a single serialized resource, and v is not needed until the first attn@v
    nc.sync.dma_start(scr[2, :], qkv_sb[6:9, :])
    nc.scalar.dma_start(nats[:, 96:144], _r(scr[2, :]))

    # vplus quads [1, v0, v1, v2] per chunk; built on the (otherwise idle) GPSIMD
    vplus = sb.tile([128, 64], F32R)
    nc.gpsimd.tensor_copy(vplus.rearrange("p (c q) -> p c q", q=4)[:, :, 0:1], onesq[:].unsqueeze(-1))
    for g in range(4):
        nc.gpsimd.tensor_copy(
            vplus.rearrange("p (c q) -> p c q", q=4)[:, 4 * g : 4 * (g + 1), 1:4],
            nats[:, 96 + 12 * g : 96 + 12 * (g + 1)].rearrange("p (c d) -> p c d", d=3),
        )

    # q^T / k^T in u-order via PE transposes of natural chunks.  Only the chunks
    # needed by round 0 are produced up front; the rest are interleaved into the
    # main loop's idle PE slots (writing to spare bank regions of the round's
    # PSUM tile after the exp has read it).
    qT_u = sb.tile([3, S], F32R)
    kT_u = sb.tile([3, S], F32R)

    def transpose_group_mm(src_off, grp, ps_region):
        for ci in range(4):
            c = 4 * grp + ci
            nc.tensor.transpose(
                _r(ps_region[0:3, 128 * ci : 128 * (ci + 1)]),
                nats[:, src_off + 3 * c : src_off + 3 * (c + 1)],
                ident[:],
            )

    def transpose_group_copy(dst, grp, ps_region, eng=None):
        if eng is None:
            nc.vector.tensor_copy(dst[:, SQ * grp : SQ * (grp + 1)], ps_region[0:3, :])
        else:
            eng.copy(dst[:, SQ * grp : SQ * (grp + 1)], ps_region[0:3, :])

    def transpose_group(dst, src_off, grp, ps_region, eng=None):
        transpose_group_mm(src_off, grp, ps_region)
        transpose_group_copy(dst, grp, ps_region, eng=eng)

    # ---------------- main attention loop (software-pipelined) ----------------
    # Rounds of <=3 t-chunks (the PSUM tiles are 3 banks); the attn@[1|v]
    # accumulation lives in its own 1-bank PSUM accumulator per s-chunk, so the
    # only cross-round serialization is mm1(next) -> exp: ACT runs back-to-back.
    # The first two rounds are 2 chunks wide: their exp leaves PSUM bank 2 free,
    # which hosts in-loop transposes without any wait on the exp.
    # acc rows: [denom, o0, o1, o2], cols in u-order of s.
    ROUND_CHUNKS = [(0, 1), (2, 3), (4, 5, 6), (7, 8, 9), (10, 11, 12), (13, 14, 15)]
    NR = len(ROUND_CHUNKS)
    recip = sb.tile([1, S], F32R)
    bc_sb = sb.tile([4, S], F32R)
    outv = sb.tile([4, S], F32)
    accs = [accp.tile([4, SQ], F32, tag=f"acc{j % 2}", name=f"acc_j{j}") for j in range(4)]

    def mm1(g):
        j, r = divmod(g, NR)
        pst = pst_of(g)
        for i, c in enumerate(ROUND_CHUNKS[r]):
            nc.tensor.matmul(
                pst[:, SQ * i : SQ * (i + 1)],
                lhsT=kT_u[:, 128 * c : 128 * (c + 1)],
                rhs=qT_u[:, SQ * j : SQ * (j + 1)],
                start=True,
                stop=True,
            )

    # remaining transpose groups ride the idle PE slots: PE work at round g
    # (into the free bank 2 on the 2-wide rounds, else into bank 0 after the
    # exp's read); the PSUM->SBUF copy early in round g+1 (it overlaps that
    # round's exp); the consuming mm1 issues one or more rounds later.
    # k3 uses the free bank of 2-wide round 0; the q-group for s-chunk j+1 is
    # transposed into accs[j+1] itself (idle until its first start=True mm2
    # overwrites it) -> no wait on any exp and no PSUM-bank conflict at all
    late_groups = {0: (kT_u, 48, 3), 4: (qT_u, 0, 1),
                   10: (qT_u, 0, 2), 16: (qT_u, 0, 3)}

    def late_region(g):
        return accs[g // NR + 1][0:4, :]

    def epilogue(j, bc_ps=None):
        # ---- per-s-chunk normalization, off the ACT critical path ----
        with nc.allow_low_precision(reason="float32r is 4-byte"):
            nc.vector.reciprocal(recip[:, SQ * j : SQ * (j + 1)], _r(accs[j][0:1, :]))
        if bc_ps is None:
            # mid-loop: broadcast on the idle GPSIMD
            bc = bc_sb[0:4, SQ * j : SQ * (j + 1)]
            nc.gpsimd.partition_broadcast(bc, recip[:, SQ * j : SQ * (j + 1)])
        else:
            # final chunk: PE is idle by now and its broadcast matmul is faster
            bc = bc_ps[0:4, :]
            nc.tensor.matmul(
                bc, lhsT=ones4[:], rhs=recip[:, SQ * j : SQ * (j + 1)],
                start=True, stop=True,
            )
        # normalization multiply fused with the u -> true-order un-permute of s:
        # outv[p, 16*pp + (4j+cc)] = acc[p, 128*cc + pp] * recip[...]
        nc.vector.tensor_mul(
            outv.rearrange("p (pp c) -> p pp c", c=NCH)[:, :, 4 * j : 4 * (j + 1)],
            accs[j][0:4, :].rearrange("p (c pp) -> p pp c", pp=128),
            bc.rearrange("p (c pp) -> p pp c", pp=128),
        )

    # prologue transpose groups: q0/k0 gate round 0; k1/k2/k3 run behind
    # mm1(0) on the in-order PE (they execute during the first exps), each in
    # a PSUM region whose next writer is late enough to hide the copy
    transpose_group(qT_u, 0, 0, psA[:, 0:SQ])
    # k0's copy rides the idle ACT so it runs in parallel with q0's DVE copy
    # instead of behind it (both gate mm1(0))
    transpose_group(kT_u, 48, 0, psB[:, 0:SQ], eng=nc.scalar)
    mm1(0)
    transpose_group(kT_u, 48, 1, psB[:, SQ : 2 * SQ])
    transpose_group(kT_u, 48, 2, psB[:, 2 * SQ : 3 * SQ])

    def mm2(g, e_t):
        j, r = divmod(g, NR)
        for i, c in enumerate(ROUND_CHUNKS[r]):
            nc.tensor.matmul(
                accs[j][0:4, :],
                lhsT=vplus[:, 4 * c : 4 * (c + 1)],
                rhs=e_t[:, SQ * i : SQ * (i + 1)],
                start=(r == 0 and i == 0),
                stop=(r == NR - 1 and i == len(ROUND_CHUNKS[r]) - 1),
            )

    # mm2 of round g is issued one round LATE (in body g+1): every matmul in
    # the PE stream then has its wait already satisfied at dispatch, so the
    # in-order PE never stalls between consecutive mm1 groups and the ACT
    # (exp) runs back-to-back even across short rounds and s-chunk boundaries.
    prev_e = None
    for g in range(4 * NR):
        j, r = divmod(g, NR)
        pst = pst_of(g)
        width = SQ * len(ROUND_CHUNKS[r])
        e_t = es.tile([128, 3 * SQ], F32R)
        nc.scalar.activation(
            e_t[:, 0:width], pst[:, 0:width],
            mybir.ActivationFunctionType.Exp, scale=INV_SCALE,
        )
        if g - 1 in late_groups:
            dst, off, grp = late_groups[g - 1]
            transpose_group_copy(dst, grp, late_region(g - 1))
        # next round's qk matmuls are independent of this exp: issue them first
        # so the in-order PE stream overlaps the exp (keeps ACT back-to-back)
        if g + 1 < 4 * NR:
            mm1(g + 1)
        if g >= 1:
            mm2(g - 1, prev_e)
        if r == 0 and j >= 1:
            epilogue(j - 1)
        if g in late_groups:
            dst, off, grp = late_groups[g]
            transpose_group_mm(off, grp, late_region(g))
        prev_e = e_t

    mm2(4 * NR - 1, prev_e)
    epilogue(3)
    nc.sync.dma_start(out_dram.ap(), outv[1:4, :])


_NC_CACHE = None


def _get_program():
    global _NC_CACHE
    if _NC_CACHE is None:
        _NC_CACHE = build_program()
    return _NC_CACHE


def kernel(x1, query, key_w, value, dropout_p=0):
    x1 = np.asarray(x1, dtype=np.float32)
    query = np.asarray(query, dtype=np.float32)
    key_w = np.asarray(key_w, dtype=np.float32)
    value = np.asarray(value, dtype=np.float32)

    in_maps = []
    for h in range(H):
        w9t = np.ascontiguousarray(
            np.concatenate([query[h], key_w[h], value[h]], axis=0).T
        )  # (3, 9)
        in_maps.append({"xt": np.ascontiguousarray(x1[h].T), "wt": w9t})

    # The axon terminal very occasionally drops a worker mid-execute
    # (NRT_EXEC_UNIT_UNRECOVERABLE); the kernel itself is deterministic, so
    # retry once with a freshly built program before giving up.
    global _NC_CACHE
    last_err = None
    for attempt in range(2):
        try:
            nc = _get_program()
            res = bass_utils.run_bass_kernel_spmd(nc, in_maps, core_ids=list(range(H)))
            return np.stack([res.results[h]["out"] for h in range(H)])
        except Exception as e:  # noqa: BLE001 - transient runtime faults only
            last_err = e
            _NC_CACHE = None
            import time as _time

            _time.sleep(5.0)
    raise last_err



# revision 10
# speedup vs baseline: 1.0234x; 1.0234x over previous
"""Trainium2 Bass kernel for nn_Model_39676907886571 (per-head attention, S=2048, d=3).

Math (per head h, fully head/data parallel, one head per NeuronCore):
  q_mat = query[h] @ x[h].T          (3, S)   -> q = q_mat viewed row-major as (S, 3)
  k_mat, v_mat likewise (the reshape is a memory-reinterpreting view, not a transpose)
  attn  = softmax(q @ k.T / sqrt(3)) (S, S)
  out   = (attn @ v).T               (3, S)

Device strategy (all on-chip; S x S attention matrix never touches HBM):
  * qT/kT/vT (3, S) in TRUE s-order are built directly by 5 PE matmuls from a
    host-prepared double-shifted input layout X18 (18, 684) plus combined
    stationaries: the row-major (S,3) "view" of q_mat turns into a stride-3
    re-read of x, which the host folds into X18's column layout; the p-row
    crossings at s=2048/4096 are handled by two row-groups (shift 0/+1) inside
    each stationary and two tiny (27,2) boundary matmuls.  ~2050 PE rows total,
    no DRAM bounce, no transposes, and no output un-permute (everything stays
    in true order).
  * E^T = exp(k-chunk^T @ qT / sqrt(3)) keeps keys on partitions; attn @ [1|v]
    accumulates in PSUM via quad stationaries [1, v0, v1, v2]; the softmax
    denominator falls out of the ones column.
  * v-nat quads come from 16 tiny PE matmuls (vT-chunk @ [I3|0]) into a spare
    PSUM region, then one strided DVE copy per 4-chunk group.
  * ACT (exp of the 2048^2 matrix, ~31.7us) is the bottleneck engine; the loop
    is software-pipelined exactly as before (mm1 of round g+1 issues before
    mm2 of round g) so ACT runs back-to-back.  The final group ends with a
    1-chunk round so the tail exp is short.
  * fp32r restrictions honored: even free sizes, even + 8B-aligned psum dst
    offsets, partition bases 0/32/64 (q/k/v stacked at psum partitions
    0:3/32:35/64:67 by the construction stationaries).
"""

import numpy as np
from contextlib import ExitStack

import concourse.bass as bass
import concourse.tile as tile
from concourse import bacc, mybir
from concourse import bass_utils

F32 = mybir.dt.float32
F32R = mybir.dt.float32r

H, S, D = 8, 2048, 3
SQ = 512
INV_SCALE = float(1.0 / np.sqrt(3.0))

STD = [(0, 1), (2, 3), (4, 5, 6), (7, 8, 9), (10, 11, 12), (13, 14, 15)]
LAST = [(0, 1), (2, 3), (4, 5, 6), (7, 8, 9), (10, 11, 12), (13, 14), (15,)]
# group 0 is all-2-wide: psA/psB bank 2 stay free of attention traffic for the
# whole group, hosting the construction blocks with relaxed copy deadlines
G08 = [(0, 1), (2, 3), (4, 5), (6, 7), (8, 9), (10, 11), (12, 13), (14, 15)]
GROUPS = [G08, STD, STD, LAST]
ROUNDS = []  # (j, r_in_group, chunks)
for _j, _grp in enumerate(GROUPS):
    for _r, _ch in enumerate(_grp):
        ROUNDS.append((_j, _r, _ch))
NRT = len(ROUNDS)

# construction sub-matmuls: (dst_t0, N, rhs_kind, rhs_a)
#   rhs_kind: 0 -> X18[:, a:a+N], 1 -> XBA, 2 -> XBB
# block b covers dst t in [512b, 512(b+1))
CONSTR = {
    0: [(0, 512, 0, 0)],
    1: [(512, 170, 0, 512), (682, 2, 1, 0), (684, 340, 0, 1)],
    2: [(1024, 340, 0, 341), (1364, 2, 2, 0), (1366, 170, 0, 0)],
    3: [(1536, 512, 0, 170)],
}
# which main stationary each X18 sub-matmul uses (by dst range)
def _main_st(t0):
    if t0 < 682:
        return 0  # ST1
    if t0 < 1366:
        return 1  # ST2
    return 2  # ST4


def _r(ap):
    """Bitcast an fp32 AP to float32r (same bits)."""
    return ap.bitcast(F32R)


def build_program():
    nc = bacc.Bacc("TRN2", num_devices=H, debug=False)
    x18_dram = nc.dram_tensor("x18", (18, 684), F32, kind="ExternalInput")
    stm_dram = nc.dram_tensor("stm", (18, 3 * 67), F32, kind="ExternalInput")
    stf_dram = nc.dram_tensor("stf", (27, 2 * 67), F32, kind="ExternalInput")
    xbf_dram = nc.dram_tensor("xbf", (27, 4), F32, kind="ExternalInput")
    out_dram = nc.dram_tensor("out", (3, S), F32, kind="ExternalOutput")

    with tile.TileContext(nc) as tc, ExitStack() as ctx:
        consts = ctx.enter_context(tc.tile_pool(name="consts", bufs=1))
        sb = ctx.enter_context(tc.tile_pool(name="sb", bufs=1))
        es = ctx.enter_context(tc.tile_pool(name="es", bufs=4))
        ping = ctx.enter_context(tc.tile_pool(name="ping", bufs=1, space="PSUM"))
        accp = ctx.enter_context(tc.tile_pool(name="accp", bufs=1, space="PSUM"))

        # ---------------- constants ----------------
        identf = consts.tile([128, 128], F32)
        from concourse.masks import make_identity

        make_identity(nc, identf)
        ident = consts.tile([128, 128], F32R)
        nc.vector.tensor_copy(ident[:], identf[:])
        ident4 = consts.tile([3, 4], F32R)  # [I3 | 0]
        nc.vector.tensor_copy(ident4[:], identf[0:3, 0:4])
        onesf = consts.tile([128, 64], F32)
        nc.vector.memset(onesf, 1.0)
        ones4 = consts.tile([1, 4], F32R)
        nc.vector.tensor_copy(ones4[:], onesf[0:1, 0:4])
        # quads pre-filled with ones; cols 4c+1..3 overwritten with v later
        quads = sb.tile([128, 64], F32R)
        nc.vector.tensor_copy(quads[:], onesf[:])
        # prewarm the ACT exp table
        warm = consts.tile([1, 1], F32)
        nc.scalar.activation(warm[:], onesf[0:1, 0:1], mybir.ActivationFunctionType.Exp)

        # ---------------- input DMAs (two HWDGE queues) ----------------
        x18 = sb.tile([18, 684], F32R)
        stm = sb.tile([18, 3 * 67], F32R)
        stf = sb.tile([27, 2 * 67], F32R)
        xbf = sb.tile([27, 4], F32R)
        nc.sync.dma_start(x18[:, 0:342], _r(x18_dram.ap()[:, 0:342]))
        nc.scalar.dma_start(stm[:], _r(stm_dram.ap()))
        nc.scalar.dma_start(stf[:], _r(stf_dram.ap()))
        nc.scalar.dma_start(xbf[:], _r(xbf_dram.ap()))
        nc.scalar.dma_start(x18[:, 342:684], _r(x18_dram.ap()[:, 342:684]))

        # ---------------- tiles ----------------
        qT = sb.tile([3, S], F32R)
        kT = sb.tile([3, S], F32R)
        vT = sb.tile([3, S], F32R)
        recip = sb.tile([1, S], F32R)
        bc_sb = sb.tile([4, S], F32R)
        outv = sb.tile([4, S], F32)

        psA = ping.tile([128, 3 * SQ], F32, tag="A")
        psB = ping.tile([128, 3 * SQ], F32, tag="B")
        accs = [accp.tile([128, SQ], F32, tag=f"acc{j % 2}", name=f"acc_j{j}") for j in range(4)]
        pst_of = lambda gi: psA if gi % 2 == 0 else psB

        # construction psum slots per block: (tile, col_offset).  psA/psB bank
        # 2 is untouched by attention until group 1 round 2/3 (~14us); accs[0]
        # rows 32:35 / 64:67 are never written by mm2 (rows 0:4 only), and its
        # q rows 0:3 are copied before the two-rounds-late mm2(0) executes.
        slot_of = {0: (psA, 0), 1: (psA, 1024), 2: (psB, 1024), 3: (accs[0], 0)}
        tp_reg = accs[1]  # v-transpose target: acc1 cols 0:64 (dead until mm2 j=1)

        # PE warm-up during the input DMAs (results dead; round 0 overwrites)
        for _w in range(8):
            nc.tensor.transpose(_r(psB[0:128, 0:128]), ident[:], ident[:])

        def constr(b):
            pt, off = slot_of[b]
            for t0, n, kind, a in CONSTR[b]:
                dst = pt[0:67, off + (t0 - 512 * b) : off + (t0 - 512 * b) + n]
                if kind == 0:
                    st = stm[:, 67 * _main_st(t0) : 67 * (_main_st(t0) + 1)]
                    nc.tensor.matmul(dst, lhsT=st, rhs=x18[:, a : a + n], start=True, stop=True)
                else:
                    st = stf[:, 0:67] if kind == 1 else stf[:, 67:134]
                    xb = xbf[:, 0:2] if kind == 1 else xbf[:, 2:4]
                    nc.tensor.matmul(dst, lhsT=st, rhs=xb, start=True, stop=True)

        def copy_block(dst, row0, b, eng):
            pt, off = slot_of[b]
            eng(dst[:, 512 * b : 512 * (b + 1)], pt[row0 : row0 + 3, off : off + 512])

        def tp(c):
            nc.tensor.matmul(
                tp_reg[0:128, 4 * c : 4 * c + 4],
                lhsT=vT[:, 128 * c : 128 * (c + 1)],
                rhs=ident4[:],
                start=True,
                stop=True,
            )

        def quad_copy(g):
            nc.vector.tensor_copy(
                quads.rearrange("p (c q) -> p c q", q=4)[:, 4 * g : 4 * (g + 1), 1:4],
                tp_reg[:, 16 * g : 16 * g + 16].rearrange("p (c q) -> p c q", q=4)[:, :, 0:3],
            )

        def mm1(gi):
            j, _, chunks = ROUNDS[gi]
            pst = pst_of(gi)
            for i, c in enumerate(chunks):
                nc.tensor.matmul(
                    pst[:, SQ * i : SQ * (i + 1)],
                    lhsT=kT[:, 128 * c : 128 * (c + 1)],
                    rhs=qT[:, SQ * j : SQ * (j + 1)],
                    start=True,
                    stop=True,
                )

        def mm2(gi, e_t):
            j, r, chunks = ROUNDS[gi]
            grp = GROUPS[j]
            for i, c in enumerate(chunks):
                nc.tensor.matmul(
                    accs[j][0:4, :],
                    lhsT=quads[:, 4 * c : 4 * (c + 1)],
                    rhs=e_t[:, SQ * i : SQ * (i + 1)],
                    start=(r == 0 and i == 0),
                    stop=(r == len(grp) - 1 and i == len(chunks) - 1),
                )

        def epilogue(j, bc_ps=None):
            blk = slice(SQ * j, SQ * (j + 1))
            with nc.allow_low_precision(reason="float32r is 4-byte"):
                nc.vector.reciprocal(recip[:, blk], _r(accs[j][0:1, :]))
            bc = bc_sb[0:4, blk]
            nc.gpsimd.partition_broadcast(bc, recip[:, blk])
            nc.vector.tensor_mul(outv[:, blk], accs[j][0:4, :], bc)
            # per-chunk output DMA (true order -> contiguous), alternate queues
            eng = nc.sync if j % 2 == 0 else nc.scalar
            eng.dma_start(out_dram.ap()[:, blk], outv[1:4, blk])

        # ---------------- prologue ----------------
        constr(0)
        copy_block(qT, 0, 0, nc.vector.tensor_copy)   # gates mm1(0), DVE
        copy_block(kT, 32, 0, nc.scalar.copy)         # ACT (idle until exp 0)
        copy_block(vT, 64, 0, nc.scalar.copy)         # ACT (feeds tp c0-3)
        mm1(0)

        # PE extras injected into loop bodies (in-order PE rides exp windows).
        extras = {
            0: lambda: (constr(1), constr(3)),
            1: lambda: (constr(2), [tp(c) for c in range(0, 4)]),
            3: lambda: [tp(c) for c in range(4, 8)],
            4: lambda: [tp(c) for c in range(8, 12)],
            5: lambda: [tp(c) for c in range(12, 16)],
        }
        # DVE copies injected per body, ordered by need time: kT block b gates
        # mm1 of the rounds covering its chunks; qT block j only gates group j;
        # q of block 3 must leave accs[0] rows 0:3 before mm2(0) executes.
        cpv = nc.vector.tensor_copy
        dve_extras = {
            0: lambda: copy_block(kT, 32, 1, cpv),
            1: lambda: (quad_copy(0), copy_block(qT, 0, 3, cpv)),
            2: lambda: (copy_block(kT, 32, 2, cpv), copy_block(vT, 64, 1, cpv),
                        copy_block(kT, 32, 3, cpv)),
            3: lambda: (quad_copy(1), copy_block(vT, 64, 2, cpv)),
            4: lambda: (quad_copy(2), copy_block(vT, 64, 3, cpv)),
            5: lambda: quad_copy(3),
            6: lambda: copy_block(qT, 0, 1, cpv),
            8: lambda: copy_block(qT, 0, 2, cpv),
        }

        # ---------------- main loop ----------------
        # mm2 of round g is issued TWO rounds late (body g+2): every PE matmul
        # has its waits satisfied at dispatch and the construction-block psum
        # slots (incl. accs[0]) survive until their copies are scheduled.
        e_hist = [None, None]
        for gi in range(NRT):
            j, r, chunks = ROUNDS[gi]
            width = SQ * len(chunks)
            pst = pst_of(gi)
            e_t = es.tile([128, 3 * SQ], F32R)
            nc.scalar.activation(
                e_t[:, 0:width], pst[:, 0:width],
                mybir.ActivationFunctionType.Exp, scale=INV_SCALE,
            )
            if gi + 1 < NRT:
                mm1(gi + 1)
            if gi in extras:
                extras[gi]()
            if gi >= 2:
                mm2(gi - 2, e_hist[0])
            if r == 1 and j >= 1:
                epilogue(j - 1)
            if gi in dve_extras:
                dve_extras[gi]()
            e_hist = [e_hist[1], e_t]

        mm2(NRT - 2, e_hist[0])
        mm2(NRT - 1, e_hist[1])
        epilogue(3, bc_ps=psB)

    nc.compile()
    return nc


def _host_tables(x1h, Wq, Wk, Wv):
    """Host-side layout prep: X18 double-shifted input + combined stationaries."""
    flat = np.concatenate([x1h.reshape(-1), np.zeros(12, np.float32)])
    XS = np.ascontiguousarray(flat.reshape(684, 9).T)
    X18 = np.zeros((18, 684), np.float32)
    X18[0:9, 0:683] = XS[:, 1:684]  # G0 rows: shift +1
    X18[9:18, :] = XS               # G1 rows: shift 0

    Ws = {0: Wq, 32: Wk, 64: Wv}
    ST1 = np.zeros((18, 67), np.float32)
    ST2 = np.zeros((18, 67), np.float32)
    ST4 = np.zeros((18, 67), np.float32)
    for base, W in Ws.items():
        for cc in range(3):
            ST1[9 + 3 * cc : 12 + 3 * cc, base + cc] = W[0]
        for cc, r, g in [(0, 1, 1), (1, 2, 1), (2, 0, 0)]:
            ST2[9 * g + 3 * r : 9 * g + 3 * r + 3, base + cc] = W[1]
        for cc, r, g in [(0, 2, 1), (1, 0, 0), (2, 1, 0)]:
            ST4[9 * g + 3 * r : 9 * g + 3 * r + 3, base + cc] = W[2]

    FIXA = {(0, 0): (0, 0, 682), (0, 1): (0, 1, 682), (0, 2): (1, 0, 0),
            (1, 0): (1, 1, 0), (1, 1): (1, 2, 0), (1, 2): (1, 0, 1)}
    FIXB = {(0, 0): (1, 1, 681), (0, 1): (1, 2, 681), (0, 2): (1, 0, 682),
            (1, 0): (1, 1, 682), (1, 1): (2, 0, 0), (1, 2): (2, 1, 0)}
    STA = np.zeros((27, 67), np.float32)
    STB = np.zeros((27, 67), np.float32)
    XBA = np.zeros((27, 2), np.float32)
    XBB = np.zeros((27, 2), np.float32)
    for table, STt, XBt in ((FIXA, STA, XBA), (FIXB, STB, XBB)):
        for (col, cc), (p, r, tau) in table.items():
            rows = slice(9 * cc + 3 * r, 9 * cc + 3 * r + 3)
            XBt[rows, col] = XS[3 * r : 3 * r + 3, tau]
            for base, W in Ws.items():
                STt[rows, base + cc] = W[p]

    stm = np.concatenate([ST1, ST2, ST4], axis=1)          # (18, 201)
    stf = np.concatenate([STA, STB], axis=1)               # (27, 134)
    xbf = np.concatenate([XBA, XBB], axis=1)               # (27, 4)
    return {"x18": X18, "stm": np.ascontiguousarray(stm),
            "stf": np.ascontiguousarray(stf), "xbf": np.ascontiguousarray(xbf)}


_NC_CACHE = None


def _get_program():
    global _NC_CACHE
    if _NC_CACHE is None:
        _NC_CACHE = build_program()
    return _NC_CACHE


def kernel(x1, query, key_w, value, dropout_p=0):
    x1 = np.asarray(x1, dtype=np.float32)
    query = np.asarray(query, dtype=np.float32)
    key_w = np.asarray(key_w, dtype=np.float32)
    value = np.asarray(value, dtype=np.float32)

    in_maps = [_host_tables(x1[h], query[h], key_w[h], value[h]) for h in range(H)]

    # The axon terminal very occasionally drops a worker mid-execute
    # (NRT_EXEC_UNIT_UNRECOVERABLE); the kernel itself is deterministic, so
    # retry once with a freshly built program before giving up.
    global _NC_CACHE
    last_err = None
    for attempt in range(2):
        try:
            nc = _get_program()
            res = bass_utils.run_bass_kernel_spmd(nc, in_maps, core_ids=list(range(H)))
            return np.stack([res.results[h]["out"] for h in range(H)])
        except Exception as e:  # noqa: BLE001 - transient runtime faults only
            last_err = e
            _NC_CACHE = None
            import time as _time

            _time.sleep(5.0)
    raise last_err
